# revision 1
# baseline (speedup 1.0000x reference)
"""Trainium2 Bass kernel for nn_AttentionLayer (attention pooling over time).

Math (per sample b):
    logits[t] = u . tanh(X[b] @ W)[t]     # (T,)
    att       = softmax_t(logits)
    out[b]    = sum_t att[t] * X[b, t, :] # (D,)

Strategy:
  - Data-parallel over batch across 8 NeuronCores (B=64 -> 8 samples/core).
  - tanh bounds |logit| <= sum|u| < 5, so softmax needs NO max subtraction:
    p[t] = exp(logit[t]) is safe in fp32.  That removes the softmax barrier
    and allows a single streaming pass over X with PSUM accumulation of both
    sum_t p[t]*x[t] and sum_t p[t]; one divide per sample at the end.
  - The X@W matmul contracts over d, so it needs X^T (d on partitions); the
    weighted sum contracts over t, so it needs X natural (t on partitions).
    The host pre-casts X to bf16 in BOTH layouts; total HBM bytes per core
    equal one fp32 pass of X, and no on-chip transpose is needed.
  - All matmuls bf16 (1 cycle/row on PE) with fp32 PSUM accumulation.

Per supertile of 512 t-rows:
    DMA  : x [128,4,256] bf16 (natural), xt [128,2,512] bf16 (transposed)
    PE   : xw = W^T @ X^T -> psum [c=128(pad), t=512]      (2 mm, accum)
    ACT  : th = tanh(xw) -> sbuf bf16
    PE   : logits chunk: th[:,s*128:...]^T @ u -> pacc[:, s]  (4 mm, 1 group)
    ACT  : p = exp(pacc) -> sbuf bf16
    PE   : sacc[1,4]  += ones^T @ p                      (per-sample group)
    PE   : oacc[1,256] += p[:,s]^T @ x[:,s,:]  (4 mm, per-sample group)
Per sample: s = sum(sacc); out_row = oacc / s; DMA out.
"""

import numpy as np
import ml_dtypes

B, T, D, CTX = 64, 4096, 256, 100
NCORES = 8
BPC = B // NCORES          # samples per core
CP = 128                   # context dim padded to 128 (W/u zero-padded)
TSUP = 512                 # t-rows per supertile
BF16 = ml_dtypes.bfloat16

_NC_CACHE: dict = {}


def build_nc(bpc=BPC, t_total=T):
    """Build (and cache) the Bass graph for one core's shard."""
    key = (bpc, t_total)
    if key in _NC_CACHE:
        return _NC_CACHE[key]

    from contextlib import ExitStack
    import concourse.bass as bass
    import concourse.tile as tile
    from concourse import bacc, mybir

    nsup = t_total // TSUP
    nsub = TSUP // 128

    nc = bacc.Bacc("TRN2", target_bir_lowering=False, debug=False)
    x = nc.declare_dram_parameter("x", [bpc, t_total, D], mybir.dt.bfloat16,
                                  isOutput=False)
    xt = nc.declare_dram_parameter("xt", [bpc, D, t_total], mybir.dt.bfloat16,
                                   isOutput=False)
    w = nc.declare_dram_parameter("w", [D, CP], mybir.dt.bfloat16,
                                  isOutput=False)
    u = nc.declare_dram_parameter("u", [CP, 1], mybir.dt.bfloat16,
                                  isOutput=False)
    out = nc.declare_dram_parameter("out", [bpc, D], mybir.dt.float32,
                                    isOutput=True)

    FP32 = mybir.dt.float32
    BF = mybir.dt.bfloat16
    PSUM = bass.MemorySpace.PSUM
    AF = mybir.ActivationFunctionType

    with tile.TileContext(nc) as tc:
        with ExitStack() as ctx:
            const = ctx.enter_context(tc.tile_pool(name="const", bufs=1))
            xpool = ctx.enter_context(tc.tile_pool(name="x", bufs=4))
            xtpool = ctx.enter_context(tc.tile_pool(name="xt", bufs=4))
            thpool = ctx.enter_context(tc.tile_pool(name="th", bufs=3))
            ppool = ctx.enter_context(tc.tile_pool(name="p", bufs=3))
            fin = ctx.enter_context(tc.tile_pool(name="fin", bufs=2))
            xwps = ctx.enter_context(tc.tile_pool(name="xwps", bufs=2, space=PSUM))
            paps = ctx.enter_context(tc.tile_pool(name="paps", bufs=2, space=PSUM))
            oaps = ctx.enter_context(tc.tile_pool(name="oaps", bufs=2, space=PSUM))
            saps = ctx.enter_context(tc.tile_pool(name="saps", bufs=2, space=PSUM))

            # Constants: W chunked [d' , c_chunk, m], u, ones column.
            w_sb = const.tile([128, 2, CP], BF, tag="w")
            nc.sync.dma_start(w_sb[:], w.rearrange("(c p) m -> p c m", p=128))
            u_sb = const.tile([CP, 1], BF, tag="u")
            nc.sync.dma_start(u_sb[:], u[:, :])
            ones_sb = const.tile([128, 1], BF, tag="ones")
            nc.vector.memset(ones_sb[:], 1.0)

            for b in range(bpc):
                oacc = oaps.tile([1, D], FP32, tag="oacc")
                sacc = saps.tile([1, nsub], FP32, tag="sacc")
                for st in range(nsup):
                    t0 = st * TSUP
                    xn = xpool.tile([128, nsub, D], BF, tag="xn")
                    nc.sync.dma_start(
                        xn[:],
                        x[b, t0:t0 + TSUP, :].rearrange("(s p) d -> p s d", p=128))
                    xtt = xtpool.tile([128, 2, TSUP], BF, tag="xtt")
                    nc.sync.dma_start(
                        xtt[:],
                        xt[b, :, t0:t0 + TSUP].rearrange("(c p) t -> p c t", p=128))

                    # xw[c, t] = sum_d W[d, c] * X[t, d]
                    xwp = xwps.tile([128, TSUP], FP32, tag="xw")
                    for c in range(2):
                        nc.tensor.matmul(xwp[:], w_sb[:, c, :], xtt[:, c, :],
                                         start=(c == 0), stop=(c == 1))

                    th = thpool.tile([128, TSUP], BF, tag="th")
                    nc.scalar.activation(th[:], xwp[:], AF.Tanh)

                    # logits[t] = sum_c th[c, t] * u[c]  -> pacc[:, s] (t on partitions)
                    pacc = paps.tile([128, nsub], FP32, tag="pacc")
                    for s in range(nsub):
                        nc.tensor.matmul(pacc[:, s:s + 1],
                                         th[:, s * 128:(s + 1) * 128], u_sb[:],
                                         start=(s == 0), stop=(s == nsub - 1))

                    p_sb = ppool.tile([128, nsub], BF, tag="p")
                    nc.scalar.activation(p_sb[:], pacc[:], AF.Exp)

                    # sacc[0, s] += sum over the 128 t-rows of p[:, s]
                    nc.tensor.matmul(sacc[:], ones_sb[:], p_sb[:],
                                     start=(st == 0), stop=(st == nsup - 1))
                    # oacc[0, :] += p[:, s]^T @ x[:, s, :]
                    for s in range(nsub):
                        nc.tensor.matmul(oacc[:], p_sb[:, s:s + 1], xn[:, s, :],
                                         start=(st == 0 and s == 0),
                                         stop=(st == nsup - 1 and s == nsub - 1))

                # Finalize sample: out_row = oacc / sum(sacc)
                s1 = fin.tile([1, 1], FP32, tag="s1")
                nc.vector.reduce_sum(s1[:], sacc[:], axis=mybir.AxisListType.X)
                rinv = fin.tile([1, 1], FP32, tag="rinv")
                nc.vector.reciprocal(rinv[:], s1[:])
                osb = fin.tile([1, D], FP32, tag="osb")
                nc.vector.tensor_scalar_mul(osb[:], oacc[:], rinv[:])
                nc.sync.dma_start(out[b:b + 1, :], osb[:])

    nc.compile()
    _NC_CACHE[key] = nc
    return nc


def make_in_maps(X, W, u):
    """Shard + cast the full inputs for the 8 cores."""
    Wp = np.zeros((D, CP), dtype=BF16)
    Wp[:, :CTX] = np.asarray(W).astype(BF16)
    up = np.zeros((CP, 1), dtype=BF16)
    up[:CTX, :] = np.asarray(u).astype(BF16)
    X16 = np.asarray(X).astype(BF16)
    in_maps = []
    for i in range(NCORES):
        xs = np.ascontiguousarray(X16[i * BPC:(i + 1) * BPC])
        xts = np.ascontiguousarray(xs.transpose(0, 2, 1))
        in_maps.append({"x": xs, "xt": xts, "w": Wp, "u": up})
    return in_maps


# test.py sets _PROFILE=True to capture neuron-profile exec time here.
_PROFILE = False
LAST_RESULT = None


def kernel(X, W, u):
    global LAST_RESULT
    from concourse.bass_utils import run_bass_kernel_spmd

    nc = build_nc()
    in_maps = make_in_maps(X, W, u)
    res = run_bass_kernel_spmd(nc, in_maps, core_ids=list(range(NCORES)),
                               trace=_PROFILE)
    LAST_RESULT = res
    outs = [np.asarray(res.results[i]["out"], dtype=np.float32)
            for i in range(NCORES)]
    return np.concatenate(outs, axis=0)


# revision 3
# speedup vs baseline: 1.3378x; 1.3378x over previous
"""Trainium2 Bass kernel for nn_AttentionLayer (attention pooling over time).

Math (per sample b):
    logits[t] = u . tanh(X[b] @ W)[t]     # (T,)
    att       = softmax_t(logits)
    out[b]    = sum_t att[t] * X[b, t, :] # (D,)

Strategy:
  - Data-parallel over batch across 8 NeuronCores (B=64 -> 8 samples/core).
  - tanh bounds |logit| <= sum|u| < 5, so softmax needs NO max subtraction:
    p[t] = exp(logit[t]) is safe in fp32.  That removes the softmax barrier
    and allows a single streaming pass over X with PSUM accumulation of both
    sum_t p[t]*x[t] and sum_t p[t]; one divide per sample at the end.
  - The X@W matmul contracts over d, so it needs X^T (d on partitions); the
    weighted sum contracts over t, so it needs X natural (t on partitions).
    The host pre-casts X to bf16 in BOTH layouts; total HBM bytes per core
    equal one fp32 pass of X, and no on-chip transpose is needed.
  - All matmuls bf16 (1 cycle/row on PE) with fp32 PSUM accumulation.
  - DMA is issued as one 2 MiB slab per sample per layout.  The natural
    layout maps t-rows p*NS+s to partition p so each partition is one
    16 KiB contiguous run; the transposed layout is stored by the host in
    the matching permuted t-order j = s*128 + p (t = NS*p + s), so the
    logits produced from X^T columns line up partition-for-partition with
    the natural-layout subtiles used by the weighted sum.
  - The per-supertile chain xw -> tanh -> logits -> exp -> weighted-sum is
    software-pipelined two supertiles deep so PE never sits behind ACT.
"""

import numpy as np
import ml_dtypes

B, T, D, CTX = 64, 4096, 256, 100
NCORES = 8
BPC = B // NCORES          # samples per core
CP = 128                   # context dim padded to 128 (W/u zero-padded)
TSUP = 512                 # t-rows per supertile (one PSUM bank of xw)
BF16 = ml_dtypes.bfloat16

_NC_CACHE: dict = {}


def build_nc(bpc=BPC, t_total=T):
    """Build (and cache) the Bass graph for one core's shard."""
    key = (bpc, t_total)
    if key in _NC_CACHE:
        return _NC_CACHE[key]

    from contextlib import ExitStack
    import concourse.bass as bass
    import concourse.tile as tile
    from concourse import bacc, mybir

    nsup = t_total // TSUP     # supertiles per sample
    ns = t_total // 128        # t-rows per partition in the natural slab

    nc = bacc.Bacc("TRN2", target_bir_lowering=False, debug=False)
    x = nc.declare_dram_parameter("x", [bpc, t_total, D], mybir.dt.bfloat16,
                                  isOutput=False)
    xt = nc.declare_dram_parameter("xt", [bpc, D, t_total], mybir.dt.bfloat16,
                                   isOutput=False)
    w = nc.declare_dram_parameter("w", [D, CP], mybir.dt.bfloat16,
                                  isOutput=False)
    u = nc.declare_dram_parameter("u", [CP, 1], mybir.dt.bfloat16,
                                  isOutput=False)
    out = nc.declare_dram_parameter("out", [bpc, D], mybir.dt.float32,
                                    isOutput=True)

    FP32 = mybir.dt.float32
    BF = mybir.dt.bfloat16
    PSUM = bass.MemorySpace.PSUM
    AF = mybir.ActivationFunctionType

    with tile.TileContext(nc) as tc:
        with ExitStack() as ctx:
            const = ctx.enter_context(tc.tile_pool(name="const", bufs=1))
            xpool = ctx.enter_context(tc.tile_pool(name="x", bufs=3))
            xtpool = ctx.enter_context(tc.tile_pool(name="xt", bufs=3))
            thpool = ctx.enter_context(tc.tile_pool(name="th", bufs=4))
            ppool = ctx.enter_context(tc.tile_pool(name="p", bufs=4))
            fin = ctx.enter_context(tc.tile_pool(name="fin", bufs=2))
            xwps = ctx.enter_context(tc.tile_pool(name="xwps", bufs=3, space=PSUM))
            paps = ctx.enter_context(tc.tile_pool(name="paps", bufs=2, space=PSUM))
            oaps = ctx.enter_context(tc.tile_pool(name="oaps", bufs=2, space=PSUM))
            saps = ctx.enter_context(tc.tile_pool(name="saps", bufs=1, space=PSUM))

            # Constants: W chunked [d', c_chunk, m], u, ones column.
            w_sb = const.tile([128, 2, CP], BF, tag="w")
            nc.sync.dma_start(w_sb[:], w.rearrange("(c p) m -> p c m", p=128))
            u_sb = const.tile([CP, 1], BF, tag="u")
            nc.sync.dma_start(u_sb[:], u[:, :])
            ones_sb = const.tile([128, 1], BF, tag="ones")
            nc.vector.memset(ones_sb[:], 1.0)

            for b in range(bpc):
                # One DMA per layout per sample: 2 MiB slabs, long
                # contiguous runs per partition (16 KiB / 8 KiB).
                xn = xpool.tile([128, ns, D], BF, tag="xn")
                nc.sync.dma_start(xn[:],
                                  x[b].rearrange("(p s) d -> p s d", p=128))
                xtt = xtpool.tile([128, 2, t_total], BF, tag="xtt")
                nc.sync.dma_start(xtt[:],
                                  xt[b].rearrange("(c p) t -> p c t", p=128))

                oacc = oaps.tile([1, D], FP32, tag="oacc")
                sacc = saps.tile([1, nsup * 4], FP32, tag="sacc")

                xwp = [None] * nsup
                th = [None] * nsup
                pacc = [None] * nsup
                p_sb = [None] * nsup

                # Software pipeline: stage A/B at st, C/D at st-1, E at st-2.
                for st in range(nsup + 2):
                    if st < nsup:
                        j0 = st * TSUP
                        # A: xw[c, j] = sum_d W[d, c] * Xperm[j, d]
                        xwp[st] = xwps.tile([128, TSUP], FP32, tag="xw", name=f"xw{st}")
                        for c in range(2):
                            nc.tensor.matmul(xwp[st][:], w_sb[:, c, :],
                                             xtt[:, c, j0:j0 + TSUP],
                                             start=(c == 0), stop=(c == 1))
                        # B: tanh
                        th[st] = thpool.tile([128, TSUP], BF, tag="th", name=f"th{st}")
                        nc.scalar.activation(th[st][:], xwp[st][:], AF.Tanh)

                    if 1 <= st <= nsup:
                        sp = st - 1
                        # C: logits chunks -> pacc[:, s] (t on partitions)
                        pacc[sp] = paps.tile([128, 4], FP32, tag="pacc", name=f"pacc{sp}")
                        for s in range(4):
                            nc.tensor.matmul(pacc[sp][:, s:s + 1],
                                             th[sp][:, s * 128:(s + 1) * 128],
                                             u_sb[:],
                                             start=(s == 0), stop=(s == 3))
                        # D: p = exp(logits)
                        p_sb[sp] = ppool.tile([128, 4], BF, tag="p", name=f"p{sp}")
                        nc.scalar.activation(p_sb[sp][:], pacc[sp][:], AF.Exp)

                    if st >= 2:
                        sp = st - 2
                        # E: sacc[0, 4*sp+s] += sum_t p ; oacc += p^T @ x
                        nc.tensor.matmul(sacc[:, 4 * sp:4 * sp + 4],
                                         ones_sb[:], p_sb[sp][:],
                                         start=(sp == 0), stop=(sp == nsup - 1))
                        for s in range(4):
                            sg = 4 * sp + s   # global subtile: t = ns*p + sg
                            nc.tensor.matmul(oacc[:], p_sb[sp][:, s:s + 1],
                                             xn[:, sg, :],
                                             start=(sg == 0),
                                             stop=(sg == 4 * nsup - 1))

                # Finalize sample: out_row = oacc / sum(sacc)
                s1 = fin.tile([1, 1], FP32, tag="s1")
                nc.vector.reduce_sum(s1[:], sacc[:], axis=mybir.AxisListType.X)
                rinv = fin.tile([1, 1], FP32, tag="rinv")
                nc.vector.reciprocal(rinv[:], s1[:])
                osb = fin.tile([1, D], FP32, tag="osb")
                nc.vector.tensor_scalar_mul(osb[:], oacc[:], rinv[:])
                nc.sync.dma_start(out[b:b + 1, :], osb[:])

    nc.compile()
    _NC_CACHE[key] = nc
    return nc


def make_in_maps(X, W, u, ncores=NCORES):
    """Shard + cast the full inputs for the cores.

    xt is stored t-permuted: column j = s*128 + p holds X[t = NS*p + s, :],
    matching the natural slab's partition layout (see build_nc docstring).
    """
    Xf = np.asarray(X)
    bpc = Xf.shape[0] // ncores
    t_total = Xf.shape[1]
    ns = t_total // 128
    Wp = np.zeros((D, CP), dtype=BF16)
    Wp[:, :CTX] = np.asarray(W).astype(BF16)
    up = np.zeros((CP, 1), dtype=BF16)
    up[:CTX, :] = np.asarray(u).astype(BF16)
    X16 = Xf.astype(BF16)
    in_maps = []
    for i in range(ncores):
        xs = np.ascontiguousarray(X16[i * bpc:(i + 1) * bpc])
        # [b, 128p, NS s, d] -> [b, d, s, p] -> [b, D, T] with j = s*128+p
        xts = np.ascontiguousarray(
            xs.reshape(bpc, 128, ns, D).transpose(0, 3, 2, 1)
        ).reshape(bpc, D, t_total)
        in_maps.append({"x": xs, "xt": xts, "w": Wp, "u": up})
    return in_maps


# test.py sets _PROFILE=True to capture neuron-profile exec time here.
_PROFILE = False
LAST_RESULT = None


def kernel(X, W, u):
    global LAST_RESULT
    from concourse.bass_utils import run_bass_kernel_spmd

    nc = build_nc()
    in_maps = make_in_maps(X, W, u)
    res = run_bass_kernel_spmd(nc, in_maps, core_ids=list(range(NCORES)),
                               trace=_PROFILE)
    LAST_RESULT = res
    outs = [np.asarray(res.results[i]["out"], dtype=np.float32)
            for i in range(NCORES)]
    return np.concatenate(outs, axis=0)


# revision 4
# speedup vs baseline: 1.4236x; 1.0642x over previous
"""Trainium2 Bass kernel for nn_AttentionLayer (attention pooling over time).

Math (per sample b):
    logits[t] = u . tanh(X[b] @ W)[t]     # (T,)
    att       = softmax_t(logits)
    out[b]    = sum_t att[t] * X[b, t, :] # (D,)

Strategy:
  - Data-parallel over batch across 8 NeuronCores (B=64 -> 8 samples/core).
  - tanh bounds |logit| <= sum|u| < 5, so softmax needs NO max subtraction:
    p[t] = exp(logit[t]) is safe in fp32.  That removes the softmax barrier
    and allows a single streaming pass over X with PSUM accumulation of both
    sum_t p[t]*x[t] and sum_t p[t]; one divide per sample at the end.
  - The X@W matmul contracts over d, so it needs X^T (d on partitions); the
    weighted sum contracts over t, so it needs X natural (t on partitions).
    The host pre-casts X to bf16 in BOTH layouts; total HBM bytes per core
    equal one fp32 pass of X, and no on-chip transpose is needed.
  - All matmuls bf16 (1 cycle/row on PE) with fp32 PSUM accumulation.
  - DMA is issued as one 2 MiB slab per sample per layout.  The natural
    layout maps t-rows p*NS+s to partition p so each partition is one
    16 KiB contiguous run; the transposed layout is stored by the host in
    the matching permuted t-order j = s*128 + p (t = NS*p + s), so the
    logits produced from X^T columns line up partition-for-partition with
    the natural-layout subtiles used by the weighted sum.
  - The per-supertile chain xw -> tanh -> logits -> exp -> weighted-sum is
    software-pipelined two supertiles deep so PE never sits behind ACT.
"""

import numpy as np
import ml_dtypes

B, T, D, CTX = 64, 4096, 256, 100
NCORES = 8
BPC = B // NCORES          # samples per core
CP = 128                   # context dim padded to 128 (W/u zero-padded)
TSUP = 512                 # t-rows per supertile (one PSUM bank of xw)
BF16 = ml_dtypes.bfloat16

_NC_CACHE: dict = {}


def build_nc(bpc=BPC, t_total=T):
    """Build (and cache) the Bass graph for one core's shard."""
    key = (bpc, t_total)
    if key in _NC_CACHE:
        return _NC_CACHE[key]

    from contextlib import ExitStack
    import concourse.bass as bass
    import concourse.tile as tile
    from concourse import bacc, mybir

    nsup = t_total // TSUP     # supertiles per sample
    ns = t_total // 128        # t-rows per partition in the natural slab

    nc = bacc.Bacc("TRN2", target_bir_lowering=False, debug=False)
    x = nc.declare_dram_parameter("x", [bpc, t_total, D], mybir.dt.bfloat16,
                                  isOutput=False)
    xt = nc.declare_dram_parameter("xt", [bpc, D, t_total], mybir.dt.bfloat16,
                                   isOutput=False)
    w = nc.declare_dram_parameter("w", [D, CP], mybir.dt.bfloat16,
                                  isOutput=False)
    u = nc.declare_dram_parameter("u", [CP, 1], mybir.dt.bfloat16,
                                  isOutput=False)
    out = nc.declare_dram_parameter("out", [bpc, D], mybir.dt.float32,
                                    isOutput=True)

    FP32 = mybir.dt.float32
    BF = mybir.dt.bfloat16
    PSUM = bass.MemorySpace.PSUM
    AF = mybir.ActivationFunctionType

    with tile.TileContext(nc) as tc:
        with ExitStack() as ctx:
            const = ctx.enter_context(tc.tile_pool(name="const", bufs=1))
            xpool = ctx.enter_context(tc.tile_pool(name="x", bufs=4))
            xtpool = ctx.enter_context(tc.tile_pool(name="xt", bufs=4))
            thpool = ctx.enter_context(tc.tile_pool(name="th", bufs=4))
            ppool = ctx.enter_context(tc.tile_pool(name="p", bufs=4))
            fin = ctx.enter_context(tc.tile_pool(name="fin", bufs=2))
            xwps = ctx.enter_context(tc.tile_pool(name="xwps", bufs=3, space=PSUM))
            paps = ctx.enter_context(tc.tile_pool(name="paps", bufs=2, space=PSUM))
            oaps = ctx.enter_context(tc.tile_pool(name="oaps", bufs=2, space=PSUM))
            saps = ctx.enter_context(tc.tile_pool(name="saps", bufs=1, space=PSUM))

            # Constants: W chunked [d', c_chunk, m], u, ones column.
            w_sb = const.tile([128, 2, CP], BF, tag="w")
            nc.gpsimd.dma_start(w_sb[:], w.rearrange("(c p) m -> p c m", p=128))
            u_sb = const.tile([CP, 1], BF, tag="u")
            nc.gpsimd.dma_start(u_sb[:], u[:, :])
            ones_sb = const.tile([128, 1], BF, tag="ones")
            nc.vector.memset(ones_sb[:], 1.0)

            for b in range(bpc):
                # One DMA per layout per sample: 2 MiB slabs, long
                # contiguous runs per partition (16 KiB / 8 KiB).
                xn = xpool.tile([128, ns, D], BF, tag="xn")
                nc.sync.dma_start(xn[:],
                                  x[b].rearrange("(p s) d -> p s d", p=128))
                xtt = xtpool.tile([128, 2, t_total], BF, tag="xtt")
                nc.sync.dma_start(xtt[:],
                                  xt[b].rearrange("(c p) t -> p c t", p=128))

                oacc = oaps.tile([1, D], FP32, tag="oacc")
                sacc = saps.tile([1, nsup * 4], FP32, tag="sacc")

                xwp = [None] * nsup
                th = [None] * nsup
                pacc = [None] * nsup
                p_sb = [None] * nsup

                # Software pipeline: stage A/B at st, C/D at st-1, E at st-2.
                for st in range(nsup + 2):
                    if st < nsup:
                        j0 = st * TSUP
                        # A: xw[c, j] = sum_d W[d, c] * Xperm[j, d]
                        xwp[st] = xwps.tile([128, TSUP], FP32, tag="xw", name=f"xw{st}")
                        for c in range(2):
                            nc.tensor.matmul(xwp[st][:], w_sb[:, c, :],
                                             xtt[:, c, j0:j0 + TSUP],
                                             start=(c == 0), stop=(c == 1))
                        # B: tanh
                        th[st] = thpool.tile([128, TSUP], BF, tag="th", name=f"th{st}")
                        nc.scalar.activation(th[st][:], xwp[st][:], AF.Tanh)

                    if 1 <= st <= nsup:
                        sp = st - 1
                        # C: logits chunks -> pacc[:, s] (t on partitions)
                        pacc[sp] = paps.tile([128, 4], FP32, tag="pacc", name=f"pacc{sp}")
                        for s in range(4):
                            nc.tensor.matmul(pacc[sp][:, s:s + 1],
                                             th[sp][:, s * 128:(s + 1) * 128],
                                             u_sb[:],
                                             start=(s == 0), stop=(s == 3))
                        # D: p = exp(logits)
                        p_sb[sp] = ppool.tile([128, 4], BF, tag="p", name=f"p{sp}")
                        nc.scalar.activation(p_sb[sp][:], pacc[sp][:], AF.Exp)

                    if st >= 2:
                        sp = st - 2
                        # E: sacc[0, 4*sp+s] += sum_t p ; oacc += p^T @ x
                        nc.tensor.matmul(sacc[:, 4 * sp:4 * sp + 4],
                                         ones_sb[:], p_sb[sp][:],
                                         start=(sp == 0), stop=(sp == nsup - 1))
                        for s in range(4):
                            sg = 4 * sp + s   # global subtile: t = ns*p + sg
                            nc.tensor.matmul(oacc[:], p_sb[sp][:, s:s + 1],
                                             xn[:, sg, :],
                                             start=(sg == 0),
                                             stop=(sg == 4 * nsup - 1))

                # Finalize sample: out_row = oacc / sum(sacc)
                s1 = fin.tile([1, 1], FP32, tag="s1")
                nc.vector.reduce_sum(s1[:], sacc[:], axis=mybir.AxisListType.X)
                rinv = fin.tile([1, 1], FP32, tag="rinv")
                nc.vector.reciprocal(rinv[:], s1[:])
                osb = fin.tile([1, D], FP32, tag="osb")
                nc.vector.tensor_scalar_mul(osb[:], oacc[:], rinv[:])
                nc.gpsimd.dma_start(out[b:b + 1, :], osb[:])

    nc.compile()
    _NC_CACHE[key] = nc
    return nc


def make_in_maps(X, W, u, ncores=NCORES):
    """Shard + cast the full inputs for the cores.

    xt is stored t-permuted: column j = s*128 + p holds X[t = NS*p + s, :],
    matching the natural slab's partition layout (see build_nc docstring).
    """
    Xf = np.asarray(X)
    bpc = Xf.shape[0] // ncores
    t_total = Xf.shape[1]
    ns = t_total // 128
    Wp = np.zeros((D, CP), dtype=BF16)
    Wp[:, :CTX] = np.asarray(W).astype(BF16)
    up = np.zeros((CP, 1), dtype=BF16)
    up[:CTX, :] = np.asarray(u).astype(BF16)
    X16 = Xf.astype(BF16)
    in_maps = []
    for i in range(ncores):
        xs = np.ascontiguousarray(X16[i * bpc:(i + 1) * bpc])
        # [b, 128p, NS s, d] -> [b, d, s, p] -> [b, D, T] with j = s*128+p
        xts = np.ascontiguousarray(
            xs.reshape(bpc, 128, ns, D).transpose(0, 3, 2, 1)
        ).reshape(bpc, D, t_total)
        in_maps.append({"x": xs, "xt": xts, "w": Wp, "u": up})
    return in_maps


# test.py sets _PROFILE=True to capture neuron-profile exec time here.
_PROFILE = False
LAST_RESULT = None


def kernel(X, W, u):
    global LAST_RESULT
    from concourse.bass_utils import run_bass_kernel_spmd

    nc = build_nc()
    in_maps = make_in_maps(X, W, u)
    res = run_bass_kernel_spmd(nc, in_maps, core_ids=list(range(NCORES)),
                               trace=_PROFILE)
    LAST_RESULT = res
    outs = [np.asarray(res.results[i]["out"], dtype=np.float32)
            for i in range(NCORES)]
    return np.concatenate(outs, axis=0)


# revision 9
# speedup vs baseline: 1.6166x; 1.1355x over previous
"""Trainium2 Bass kernel for nn_AttentionLayer (attention pooling over time).

Math (per sample b):
    logits[t] = u . tanh(X[b] @ W)[t]     # (T,)
    att       = softmax_t(logits)
    out[b]    = sum_t att[t] * X[b, t, :] # (D,)

Strategy:
  - Data-parallel over batch across 8 NeuronCores (B=64 -> 8 samples/core).
  - tanh bounds |logit| <= sum|u| < 5, so softmax needs NO max subtraction:
    p[t] = exp(logit[t]) is safe in fp32.  That removes the softmax barrier
    and allows a single streaming pass over X with PSUM accumulation of both
    sum_t p[t]*x[t] and sum_t p[t]; one divide per sample at the end.
  - The X@W matmul contracts over d, so it needs X^T (d on partitions); the
    weighted sum contracts over t, so it needs X natural (t on partitions).
    The host pre-casts X to bf16 in BOTH layouts; total HBM bytes per core
    equal one fp32 pass of X, and no on-chip transpose is needed.
  - All matmuls bf16 (1 cycle/row on PE) with fp32 PSUM accumulation.
  - DMA is issued as one 2 MiB slab per sample per layout.  The natural
    layout maps t-rows p*NS+s to partition p so each partition is one
    16 KiB contiguous run; the transposed layout is stored by the host in
    the matching permuted t-order j = s*128 + p (t = NS*p + s), so the
    logits produced from X^T columns line up partition-for-partition with
    the natural-layout subtiles used by the weighted sum.
  - The per-supertile chain xw -> tanh -> logits -> exp -> weighted-sum is
    software-pipelined two supertiles deep so PE never sits behind ACT.
"""

import numpy as np
import ml_dtypes

B, T, D, CTX = 64, 4096, 256, 100
NCORES = 8
BPC = B // NCORES          # samples per core
CP = 128                   # context dim padded to 128 (W/u zero-padded)
TSUP = 512                 # t-rows per supertile (one PSUM bank of xw)
BF16 = ml_dtypes.bfloat16

_NC_CACHE: dict = {}


def build_nc(bpc=BPC, t_total=T):
    """Build (and cache) the Bass graph for one core's shard."""
    key = (bpc, t_total)
    if key in _NC_CACHE:
        return _NC_CACHE[key]

    from contextlib import ExitStack
    import concourse.bass as bass
    import concourse.tile as tile
    from concourse import bacc, mybir

    nsup = t_total // TSUP     # supertiles per sample
    t_half = t_total // 2      # DMA slab = half a sample per layout
    nsup_h = nsup // 2         # supertiles per half-slab
    ns_h = t_half // 128       # t-rows per partition in one natural slab

    nc = bacc.Bacc("TRN2", target_bir_lowering=False, debug=False)
    x = nc.declare_dram_parameter("x", [bpc, t_total, D], mybir.dt.bfloat16,
                                  isOutput=False)
    xt = nc.declare_dram_parameter("xt", [bpc, 2, D, t_half],
                                   mybir.dt.bfloat16, isOutput=False)
    w = nc.declare_dram_parameter("w", [D, CP], mybir.dt.bfloat16,
                                  isOutput=False)
    u = nc.declare_dram_parameter("u", [CP, 1], mybir.dt.bfloat16,
                                  isOutput=False)
    out = nc.declare_dram_parameter("out", [bpc, D], mybir.dt.float32,
                                    isOutput=True)

    FP32 = mybir.dt.float32
    BF = mybir.dt.bfloat16
    PSUM = bass.MemorySpace.PSUM
    AF = mybir.ActivationFunctionType

    with tile.TileContext(nc) as tc:
        with ExitStack() as ctx:
            const = ctx.enter_context(tc.tile_pool(name="const", bufs=1))
            xpool = ctx.enter_context(tc.tile_pool(name="x", bufs=6))
            xtpool = ctx.enter_context(tc.tile_pool(name="xt", bufs=6))
            thpool = ctx.enter_context(tc.tile_pool(name="th", bufs=4))
            ppool = ctx.enter_context(tc.tile_pool(name="p", bufs=4))
            fin = ctx.enter_context(tc.tile_pool(name="fin", bufs=2))
            xwps = ctx.enter_context(tc.tile_pool(name="xwps", bufs=3, space=PSUM))
            paps = ctx.enter_context(tc.tile_pool(name="paps", bufs=2, space=PSUM))
            oaps = ctx.enter_context(tc.tile_pool(name="oaps", bufs=2, space=PSUM))
            saps = ctx.enter_context(tc.tile_pool(name="saps", bufs=1, space=PSUM))

            # Constants: W chunked [d', c_chunk, m], u, ones column.
            w_sb = const.tile([128, 2, CP], BF, tag="w")
            nc.gpsimd.dma_start(w_sb[:], w.rearrange("(c p) m -> p c m", p=128))
            u_sb = const.tile([CP, 1], BF, tag="u")
            nc.gpsimd.dma_start(u_sb[:], u[:, :])
            ones_sb = const.tile([128, 1], BF, tag="ones")
            nc.vector.memset(ones_sb[:], 1.0)

            for b in range(bpc):
                # Two 1 MiB DMA slabs per layout per sample (halves of the
                # t-range), xt first since it heads the compute pipeline.
                # Runs per partition stay long (8 KiB / 4 KiB).
                xn = [None, None]
                xtt = [None, None]
                for h in range(2):
                    xtt[h] = xtpool.tile([128, 2, t_half], BF, tag="xtt",
                                         name=f"xtt{b}_{h}")
                    nc.sync.dma_start(
                        xtt[h][:],
                        xt[b, h].rearrange("(c p) t -> p c t", p=128))
                    xn[h] = xpool.tile([128, ns_h, D], BF, tag="xn",
                                       name=f"xn{b}_{h}")
                    nc.sync.dma_start(
                        xn[h][:],
                        x[b, h * t_half:(h + 1) * t_half, :].rearrange(
                            "(p s) d -> p s d", p=128))

                oacc = oaps.tile([1, D], FP32, tag="oacc")
                sacc = saps.tile([1, nsup * 4], FP32, tag="sacc")

                xwp = [None] * nsup
                th = [None] * nsup
                pacc = [None] * nsup
                p_sb = [None] * nsup

                # Software pipeline: stage A/B at st, C/D at st-1, E at st-2.
                for st in range(nsup + 2):
                    if st < nsup:
                        h = st // nsup_h
                        j0 = (st % nsup_h) * TSUP
                        # A: xw[c, j] = sum_d W[d, c] * Xperm[j, d]
                        xwp[st] = xwps.tile([128, TSUP], FP32, tag="xw", name=f"xw{st}")
                        for c in range(2):
                            nc.tensor.matmul(xwp[st][:], w_sb[:, c, :],
                                             xtt[h][:, c, j0:j0 + TSUP],
                                             start=(c == 0), stop=(c == 1))
                        # B: tanh
                        th[st] = thpool.tile([128, TSUP], BF, tag="th", name=f"th{st}")
                        nc.scalar.activation(th[st][:], xwp[st][:], AF.Tanh)

                    if 1 <= st <= nsup:
                        sp = st - 1
                        # C: logits chunks -> pacc[:, s] (t on partitions)
                        pacc[sp] = paps.tile([128, 4], FP32, tag="pacc", name=f"pacc{sp}")
                        for s in range(4):
                            nc.tensor.matmul(pacc[sp][:, s:s + 1],
                                             th[sp][:, s * 128:(s + 1) * 128],
                                             u_sb[:],
                                             start=(s == 0), stop=(s == 3))
                        # D: p = exp(logits)
                        p_sb[sp] = ppool.tile([128, 4], BF, tag="p", name=f"p{sp}")
                        nc.scalar.activation(p_sb[sp][:], pacc[sp][:], AF.Exp)

                    if st >= 2:
                        sp = st - 2
                        # E: sacc[0, 4*sp+s] += sum_t p ; oacc += p^T @ x
                        nc.tensor.matmul(sacc[:, 4 * sp:4 * sp + 4],
                                         ones_sb[:], p_sb[sp][:],
                                         start=(sp == 0), stop=(sp == nsup - 1))
                        for s in range(4):
                            sg = 4 * sp + s   # global subtile index
                            h2, sl = sg // ns_h, sg % ns_h
                            nc.tensor.matmul(oacc[:], p_sb[sp][:, s:s + 1],
                                             xn[h2][:, sl, :],
                                             start=(sg == 0),
                                             stop=(sg == 4 * nsup - 1))

                # Finalize sample: out_row = oacc / sum(sacc)
                s1 = fin.tile([1, 1], FP32, tag="s1")
                nc.vector.reduce_sum(s1[:], sacc[:], axis=mybir.AxisListType.X)
                rinv = fin.tile([1, 1], FP32, tag="rinv")
                nc.vector.reciprocal(rinv[:], s1[:])
                osb = fin.tile([1, D], FP32, tag="osb")
                nc.vector.tensor_scalar_mul(osb[:], oacc[:], rinv[:])
                nc.gpsimd.dma_start(out[b:b + 1, :], osb[:])

    nc.compile()
    _NC_CACHE[key] = nc
    return nc


def make_in_maps(X, W, u, ncores=NCORES):
    """Shard + cast the full inputs for the cores.

    xt is stored t-permuted: column j = s*128 + p holds X[t = NS*p + s, :],
    matching the natural slab's partition layout (see build_nc docstring).
    """
    Xf = np.asarray(X)
    bpc = Xf.shape[0] // ncores
    t_total = Xf.shape[1]
    ns = t_total // 128
    Wp = np.zeros((D, CP), dtype=BF16)
    Wp[:, :CTX] = np.asarray(W).astype(BF16)
    up = np.zeros((CP, 1), dtype=BF16)
    up[:CTX, :] = np.asarray(u).astype(BF16)
    X16 = Xf.astype(BF16)
    in_maps = []
    for i in range(ncores):
        xs = np.ascontiguousarray(X16[i * bpc:(i + 1) * bpc])
        # per half: [b, h, 128p, s, d] -> [b, h, d, s, p]; j = s*128 + p
        ns_h = ns // 2
        xts = np.ascontiguousarray(
            xs.reshape(bpc, 2, 128, ns_h, D).transpose(0, 1, 4, 3, 2)
        ).reshape(bpc, 2, D, t_total // 2)
        in_maps.append({"x": xs, "xt": xts, "w": Wp, "u": up})
    return in_maps


# test.py sets _PROFILE=True to capture neuron-profile exec time here.
_PROFILE = False
LAST_RESULT = None


def kernel(X, W, u):
    global LAST_RESULT
    from concourse.bass_utils import run_bass_kernel_spmd

    nc = build_nc()
    in_maps = make_in_maps(X, W, u)
    res = run_bass_kernel_spmd(nc, in_maps, core_ids=list(range(NCORES)),
                               trace=_PROFILE)
    LAST_RESULT = res
    outs = [np.asarray(res.results[i]["out"], dtype=np.float32)
            for i in range(NCORES)]
    return np.concatenate(outs, axis=0)
